# revision 8
# baseline (speedup 1.0000x reference)
"""Trainium2 kernel: out[n, k] = prod_c vector_list[n, c] ** l_list[k, c].

Data-parallel over 8 NeuronCores: vector_list is sharded along the row
dimension; the exponent table l_list is baked into the instruction stream
host-side (exponents are small non-negative ints), so each output column
is one elementwise op (mul / square / copy / memset) reading previously
computed columns in SBUF.

Layout per core: rows are tiled [128 partitions, F rows, .] so both the
input load and the output store are fully contiguous DRAM transfers; the
per-column compute ops use stride-L views of the output tile.
"""

import math
import sys

import numpy as np

sys.path.insert(0, "/opt/trn_rl_repo")

P = 128          # SBUF partitions
F_MAX = 280      # max rows-per-partition per chunk
N_CORES = 8
GP_MULS = 5      # leaf muls offloaded from VectorE to GpSimd


def _plan(exps):
    """Codegen plan for computing all monomial columns.

    exps: list of (lx, ly, lz) int tuples.
    Returns (steps, n_scratch) where steps reference value symbols:
      ('basis', c) — input component c as a strided view
      ('out', k)   — output column k
      ('scr', i)   — scratch column i
    Step kinds:
      ('one', dst) | ('copy', dst, src) | ('square', dst, src)
      | ('mul', dst, a, b)
    """
    basis = {(1, 0, 0): ("basis", 0), (0, 1, 0): ("basis", 1), (0, 0, 1): ("basis", 2)}
    avail = {}
    steps = []
    nscr = 0

    def sub(e, u):
        return (e[0] - u[0], e[1] - u[1], e[2] - u[2])

    def lookup(t):
        if t in avail:
            return avail[t]
        if t in basis:
            return basis[t]
        return None

    def get(e):
        nonlocal nscr
        v = lookup(e)
        if v is not None:
            return v
        dst = ("scr", nscr)
        nscr += 1
        emit(dst, e)
        return dst

    def emit(dst, e):
        # Prefer an ACT square (offloads the vector engine) when e is even
        # and its half is already materialized.
        if all(c % 2 == 0 for c in e):
            h = tuple(c // 2 for c in e)
            if lookup(h) is not None:
                steps.append(("square", dst, lookup(h)))
                avail[e] = dst
                return
        # Split into two already-available factors.
        for a in list(avail.keys()) + list(basis.keys()):
            if sum(a) == 0 or sub(e, a) == e:
                continue
            b = sub(e, a)
            if min(b) < 0 or sum(b) == 0:
                continue
            va, vb = lookup(a), lookup(b)
            if va is not None and vb is not None:
                steps.append(("mul", dst, va, vb))
                avail[e] = dst
                return
        # Even split via square of a recursively built half.
        if all(c % 2 == 0 for c in e):
            h = tuple(c // 2 for c in e)
            steps.append(("square", dst, get(h)))
            avail[e] = dst
            return
        # Peel a unit factor off the largest axis.
        ax = max(range(3), key=lambda i: e[i])
        u = tuple(1 if i == ax else 0 for i in range(3))
        rest = get(sub(e, u))
        steps.append(("mul", dst, rest, basis[u]))
        avail[e] = dst

    order = sorted(range(len(exps)), key=lambda k: (sum(exps[k]), exps[k]))
    for k in order:
        e = tuple(exps[k])
        dst = ("out", k)
        if sum(e) == 0:
            steps.append(("one", dst))
            continue
        if e in avail:
            steps.append(("copy", dst, avail[e]))
            continue
        if e in basis:
            # Materialize the column, but keep reading the basis view for
            # later products (avoids a cross-op dependency on the copy).
            steps.append(("copy", dst, basis[e]))
            avail[e] = basis[e]
            continue
        emit(dst, e)
    return steps, nscr


def _build(R, exps):
    """Build the per-core Bacc graph for R rows (R divisible by P)."""
    import concourse.bass as bass  # noqa: F401  (engine types)
    import concourse.tile as tile
    from concourse import bacc, mybir

    L = len(exps)
    steps, nscr = _plan(exps)
    f32 = mybir.dt.float32

    # Engine balance (measured per-op costs at F~280: DVE mul 570ns,
    # DVE copy 274ns, ACT square/copy 705ns, GpSimd 1-input ~line rate):
    # squares -> ACT, memset/copies -> GpSimd, muls -> DVE except a few
    # leaf muls (results never read again) offloaded to GpSimd.
    used = set()
    for st in steps:
        for sym in st[2:]:
            used.add(sym)
    gp_muls = set()
    for i in reversed(range(len(steps))):
        st = steps[i]
        if len(gp_muls) >= GP_MULS:
            break
        if st[0] == "mul" and st[1] not in used:
            gp_muls.add(i)

    nc = bacc.Bacc()
    vec = nc.declare_dram_parameter("vector_list", [R, 3], f32, isOutput=False)
    out = nc.declare_dram_parameter("out", [R, L], f32, isOutput=True)

    rows_p = R // P
    n_chunks = max(1, math.ceil(rows_p / F_MAX))
    base, rem = divmod(rows_p, n_chunks)
    sizes = [base + 1] * rem + [base] * (n_chunks - rem)
    if len(sizes) >= 2:
        # Taper: split the first and last chunks so the store pipeline
        # starts earlier and drains faster.
        f, l = sizes[0], sizes[-1]
        sizes = [f // 2, f - f // 2] + sizes[1:-1] + [l // 2, l - l // 2]
        sizes = [s for s in sizes if s > 0]

    with tile.TileContext(nc) as tc:
        with (
            tc.tile_pool(name="inp", bufs=4) as inp,
            tc.tile_pool(name="outp", bufs=3) as outp,
            tc.tile_pool(name="scrp", bufs=2) as scrp,
        ):
            r0 = 0
            for F in sizes:
                rows = P * F
                tin = inp.tile([P, F, 3], f32)
                nc.scalar.dma_start(
                    tin[:],
                    vec[r0 : r0 + rows, :].rearrange("(p f) c -> p f c", p=P),
                )
                tout = outp.tile([P, F, L], f32)
                tscr = scrp.tile([P, F, nscr], f32) if nscr else None

                def ap(sym):
                    kind, i = sym
                    if kind == "basis":
                        return tin[:, :, i]
                    if kind == "out":
                        return tout[:, :, i]
                    return tscr[:, :, i]

                for i, st in enumerate(steps):
                    if st[0] == "one":
                        nc.gpsimd.memset(ap(st[1]), 1.0)
                    elif st[0] == "copy":
                        nc.gpsimd.tensor_copy(ap(st[1]), ap(st[2]))
                    elif st[0] == "square":
                        nc.scalar.square(ap(st[1]), ap(st[2]))
                    elif i in gp_muls:
                        nc.gpsimd.tensor_mul(ap(st[1]), ap(st[2]), ap(st[3]))
                    else:
                        nc.vector.tensor_mul(ap(st[1]), ap(st[2]), ap(st[3]))

                nc.sync.dma_start(
                    out[r0 : r0 + rows, :].rearrange("(p f) k -> p f k", p=P),
                    tout[:],
                )
                r0 += rows
    nc.finalize()
    return nc


_CACHE = {}
_LAST_RESULT = None  # BassKernelResults of the most recent run (for profiling)


def kernel(vector_list: np.ndarray, l_list: np.ndarray) -> np.ndarray:
    from concourse.bass_utils import run_bass_kernel_spmd

    vector_list = np.ascontiguousarray(vector_list, dtype=np.float32)
    l_list = np.asarray(l_list)
    N = vector_list.shape[0]
    L = l_list.shape[0]
    exps = tuple(tuple(int(v) for v in row) for row in l_list)

    rows_unit = N_CORES * P
    n_dev = (N // rows_unit) * rows_unit
    R = n_dev // N_CORES

    outv = np.empty((N, L), dtype=np.float32)
    if R > 0:
        key = (R, exps)
        if key not in _CACHE:
            _CACHE[key] = _build(R, exps)
        nc = _CACHE[key]
        in_maps = [
            {"vector_list": vector_list[i * R : (i + 1) * R]} for i in range(N_CORES)
        ]
        res = run_bass_kernel_spmd(nc, in_maps, core_ids=list(range(N_CORES)))
        global _LAST_RESULT
        _LAST_RESULT = res
        for i in range(N_CORES):
            outv[i * R : (i + 1) * R] = res.results[i]["out"]
    if n_dev < N:
        tail = vector_list[n_dev:]
        le = np.asarray(l_list, dtype=np.float32)
        outv[n_dev:] = np.prod(
            tail[:, None, :] ** le[None, :, :], axis=-1, dtype=np.float32
        )
    return outv


# revision 9
# speedup vs baseline: 1.2408x; 1.2408x over previous
"""Trainium2 kernel: out[n, k] = prod_c vector_list[n, c] ** l_list[k, c].

Data-parallel over 8 NeuronCores: vector_list is sharded along the row
dimension; the exponent table l_list is baked into the instruction stream
host-side (exponents are small non-negative ints), so each output column
is one elementwise op (mul / square / copy / memset) reading previously
computed columns in SBUF.

Layout per core: rows are tiled [128 partitions, F rows, .] so both the
input load and the output store are fully contiguous DRAM transfers; the
per-column compute ops use stride-L views of the output tile.
"""

import math
import sys

import numpy as np

sys.path.insert(0, "/opt/trn_rl_repo")

P = 128          # SBUF partitions
F_MAX = 280      # max rows-per-partition per chunk
N_CORES = 8
GP_MULS = 0      # leaf muls offloaded from VectorE to GpSimd (it is ~3x slower; keep 0)


def _plan(exps):
    """Codegen plan for computing all monomial columns.

    exps: list of (lx, ly, lz) int tuples.
    Returns (steps, n_scratch) where steps reference value symbols:
      ('basis', c) — input component c as a strided view
      ('out', k)   — output column k
      ('scr', i)   — scratch column i
    Step kinds:
      ('one', dst) | ('copy', dst, src) | ('square', dst, src)
      | ('mul', dst, a, b)
    """
    basis = {(1, 0, 0): ("basis", 0), (0, 1, 0): ("basis", 1), (0, 0, 1): ("basis", 2)}
    avail = {}
    steps = []
    nscr = 0

    def sub(e, u):
        return (e[0] - u[0], e[1] - u[1], e[2] - u[2])

    def lookup(t):
        if t in avail:
            return avail[t]
        if t in basis:
            return basis[t]
        return None

    def get(e):
        nonlocal nscr
        v = lookup(e)
        if v is not None:
            return v
        dst = ("scr", nscr)
        nscr += 1
        emit(dst, e)
        return dst

    def emit(dst, e):
        # Prefer an ACT square (offloads the vector engine) when e is even
        # and its half is already materialized.
        if all(c % 2 == 0 for c in e):
            h = tuple(c // 2 for c in e)
            if lookup(h) is not None:
                steps.append(("square", dst, lookup(h)))
                avail[e] = dst
                return
        # Split into two already-available factors.
        for a in list(avail.keys()) + list(basis.keys()):
            if sum(a) == 0 or sub(e, a) == e:
                continue
            b = sub(e, a)
            if min(b) < 0 or sum(b) == 0:
                continue
            va, vb = lookup(a), lookup(b)
            if va is not None and vb is not None:
                steps.append(("mul", dst, va, vb))
                avail[e] = dst
                return
        # Even split via square of a recursively built half.
        if all(c % 2 == 0 for c in e):
            h = tuple(c // 2 for c in e)
            steps.append(("square", dst, get(h)))
            avail[e] = dst
            return
        # Peel a unit factor off the largest axis.
        ax = max(range(3), key=lambda i: e[i])
        u = tuple(1 if i == ax else 0 for i in range(3))
        rest = get(sub(e, u))
        steps.append(("mul", dst, rest, basis[u]))
        avail[e] = dst

    order = sorted(range(len(exps)), key=lambda k: (sum(exps[k]), exps[k]))
    for k in order:
        e = tuple(exps[k])
        dst = ("out", k)
        if sum(e) == 0:
            steps.append(("one", dst))
            continue
        if e in avail:
            steps.append(("copy", dst, avail[e]))
            continue
        if e in basis:
            # Materialize the column, but keep reading the basis view for
            # later products (avoids a cross-op dependency on the copy).
            steps.append(("copy", dst, basis[e]))
            avail[e] = basis[e]
            continue
        emit(dst, e)
    return steps, nscr


def _build(R, exps):
    """Build the per-core Bacc graph for R rows (R divisible by P)."""
    import concourse.bass as bass  # noqa: F401  (engine types)
    import concourse.tile as tile
    from concourse import bacc, mybir

    L = len(exps)
    steps, nscr = _plan(exps)
    f32 = mybir.dt.float32

    # Engine balance (measured per-op costs at F~280: DVE mul 570ns,
    # DVE copy 274ns, ACT square/copy 705ns, GpSimd 1-input ~line rate):
    # squares -> ACT, memset/copies -> GpSimd, muls -> DVE except a few
    # leaf muls (results never read again) offloaded to GpSimd.
    used = set()
    for st in steps:
        for sym in st[2:]:
            used.add(sym)
    gp_muls = set()
    for i in reversed(range(len(steps))):
        st = steps[i]
        if len(gp_muls) >= GP_MULS:
            break
        if st[0] == "mul" and st[1] not in used:
            gp_muls.add(i)

    nc = bacc.Bacc()
    vec = nc.declare_dram_parameter("vector_list", [R, 3], f32, isOutput=False)
    out = nc.declare_dram_parameter("out", [R, L], f32, isOutput=True)

    rows_p = R // P
    n_chunks = max(1, math.ceil(rows_p / F_MAX))
    base, rem = divmod(rows_p, n_chunks)
    sizes = [base + 1] * rem + [base] * (n_chunks - rem)
    if len(sizes) >= 2:
        # Taper: split the first and last chunks so the store pipeline
        # starts earlier and drains faster.
        f, l = sizes[0], sizes[-1]
        sizes = [f // 2, f - f // 2] + sizes[1:-1] + [l // 2, l - l // 2]
        sizes = [s for s in sizes if s > 0]

    with tile.TileContext(nc) as tc:
        with (
            tc.tile_pool(name="inp", bufs=4) as inp,
            tc.tile_pool(name="outp", bufs=4) as outp,
            tc.tile_pool(name="scrp", bufs=2) as scrp,
        ):
            r0 = 0
            for F in sizes:
                rows = P * F
                tin = inp.tile([P, F, 3], f32)
                nc.scalar.dma_start(
                    tin[:],
                    vec[r0 : r0 + rows, :].rearrange("(p f) c -> p f c", p=P),
                )
                tout = outp.tile([P, F, L], f32)
                tscr = scrp.tile([P, F, nscr], f32) if nscr else None

                def ap(sym):
                    kind, i = sym
                    if kind == "basis":
                        return tin[:, :, i]
                    if kind == "out":
                        return tout[:, :, i]
                    return tscr[:, :, i]

                for i, st in enumerate(steps):
                    if st[0] == "one":
                        nc.gpsimd.memset(ap(st[1]), 1.0)
                    elif st[0] == "copy":
                        nc.scalar.copy(ap(st[1]), ap(st[2]))
                    elif st[0] == "square":
                        nc.scalar.square(ap(st[1]), ap(st[2]))
                    elif i in gp_muls:
                        nc.gpsimd.tensor_mul(ap(st[1]), ap(st[2]), ap(st[3]))
                    else:
                        nc.vector.tensor_mul(ap(st[1]), ap(st[2]), ap(st[3]))

                nc.sync.dma_start(
                    out[r0 : r0 + rows, :].rearrange("(p f) k -> p f k", p=P),
                    tout[:],
                )
                r0 += rows
    nc.finalize()
    return nc


_CACHE = {}
_LAST_RESULT = None  # BassKernelResults of the most recent run (for profiling)


def kernel(vector_list: np.ndarray, l_list: np.ndarray) -> np.ndarray:
    from concourse.bass_utils import run_bass_kernel_spmd

    vector_list = np.ascontiguousarray(vector_list, dtype=np.float32)
    l_list = np.asarray(l_list)
    N = vector_list.shape[0]
    L = l_list.shape[0]
    exps = tuple(tuple(int(v) for v in row) for row in l_list)

    rows_unit = N_CORES * P
    n_dev = (N // rows_unit) * rows_unit
    R = n_dev // N_CORES

    outv = np.empty((N, L), dtype=np.float32)
    if R > 0:
        key = (R, exps)
        if key not in _CACHE:
            _CACHE[key] = _build(R, exps)
        nc = _CACHE[key]
        in_maps = [
            {"vector_list": vector_list[i * R : (i + 1) * R]} for i in range(N_CORES)
        ]
        res = run_bass_kernel_spmd(nc, in_maps, core_ids=list(range(N_CORES)))
        global _LAST_RESULT
        _LAST_RESULT = res
        for i in range(N_CORES):
            outv[i * R : (i + 1) * R] = res.results[i]["out"]
    if n_dev < N:
        tail = vector_list[n_dev:]
        le = np.asarray(l_list, dtype=np.float32)
        outv[n_dev:] = np.prod(
            tail[:, None, :] ** le[None, :, :], axis=-1, dtype=np.float32
        )
    return outv
